# revision 16
# baseline (speedup 1.0000x reference)
"""MoE ExpertGroup kernel for Trainium2 (8 NeuronCores, expert-parallel).

Problem: E=8 experts, H=1024, I=4096, N=16384 tokens sorted by expert.
y[t] = gelu_tanh(x[t] @ w1[e(t)]) @ w2[e(t)]

Sharding: expert-parallel - core e holds expert e's weights and processes
expert e's token block (balanced routing: 2048 tokens/core). All matmul
operands are bf16 (full-rate on the PE, half the DMA traffic); y is
returned in bf16 and upcast on the host (rel err ~4e-3 vs 2e-2 budget).

Per-core structure (PE stream floor: 2048 matmuls x 512 cols @2.4GHz
= 442us; everything else is edges):
- Host pre-packs x/w1/w2 into k-major SBUF-shaped layouts so each DMA
  is ONE large contiguous-line transfer (the Sync queue issues triggers
  at only ~1.6/us, and each trigger fans out over all 16 DMA engines).
- MM1 pass 1 (blocks b0,b1): group g0 runs single-block chains (all b0,
  then all b1) with per-i-tile w1 triggers, so the first chain needs
  only 1.25MB landed (~12us) instead of 3MB (~18us). Groups g1..g7 run
  b0+b1 chains off one 1MB group tile, double buffered, prefetched one
  group ahead.
- MM2 accumulates all 32 I-tiles of a token tile into one PSUM group.
  y stored as bf16. For the very last token tile the second output half
  is drained by the Vector engine in parallel with the Scalar act to
  shorten the tail.
- 12 warmup matmuls ramp the PE clock (HAM un-throttle needs ~3.4us of
  busy) while the first DMAs land.
"""

import sys

sys.path.insert(0, "/opt/trn_rl_repo")

import numpy as np
import ml_dtypes

E = 8
H = 1024
I = 4096
N_TOK = 16384
T = N_TOK // E

P = 128
TB = 512
NB = T // TB          # 4 token blocks
HB = H // P           # 8 k-tiles (contraction for MM1)
IB = I // P           # 32 i-tiles
GI = 4                # i-tiles per w1 group
NG = IB // GI         # 8 groups
WARM = 9

_CACHE = {}


def _build():
    import concourse.bacc as bacc
    import concourse.mybir as mybir
    import concourse.tile as tile

    F32 = mybir.dt.float32
    BF16 = mybir.dt.bfloat16
    GELU = mybir.ActivationFunctionType.Gelu_apprx_tanh
    COPY = mybir.ActivationFunctionType.Copy

    nc = bacc.Bacc("TRN2", target_bir_lowering=False, debug=False, num_devices=E)

    # Host-packed layouts (see _prep_in_maps):
    #   xg [P, (b,k,c)]   : xg[p, b*HB*TB + k*TB + c] = x[b*TB+c, k*P+p]
    #   w1a [P, (il,k,c)] : w1a[p, il*HB*P + k*P + c] = w1[k*P+p, il*P+c]
    #   w1b [P, (g',k,c)] : w1b[p, g'*HB*TB + k*TB + c] = w1[k*P+p, (g'+1)*TB+c]
    #   w2h [P, (i,c)]    : w2h[p, i*H + c] = w2[i*P+p, c]
    xg = nc.dram_tensor("xg", [P, NB * HB * TB], BF16, kind="ExternalInput").ap()
    w1a = nc.dram_tensor("w1a", [P, GI * HB * P], BF16, kind="ExternalInput").ap()
    w1b = nc.dram_tensor(
        "w1b", [P, (NG - 1) * HB * TB], BF16, kind="ExternalInput"
    ).ap()
    w2h = nc.dram_tensor("w2h", [P, IB * H], BF16, kind="ExternalInput").ap()
    y = nc.dram_tensor("y", [T, H], BF16, kind="ExternalOutput").ap()

    with tile.TileContext(nc) as tc:
        with (
            tc.tile_pool(name="xp", bufs=1) as x_pool,
            tc.tile_pool(name="w1ap", bufs=1) as w1a_pool,
            tc.tile_pool(name="w1p", bufs=2) as w1_pool,
            tc.tile_pool(name="w2p", bufs=1) as w2_pool,
            tc.tile_pool(name="hp", bufs=1) as h_pool,
            tc.tile_pool(name="yp", bufs=4) as y_pool,
            tc.tile_pool(name="ph", bufs=4, space="PSUM") as ph_pool,
            tc.tile_pool(name="pyA", bufs=2, space="PSUM") as pyA_pool,
            tc.tile_pool(name="pyB", bufs=2, space="PSUM") as pyB_pool,
        ):
            # warmup first: PE busy while the first DMAs land (HAM ramp)
            warm = y_pool.tile([P, TB], BF16, tag="warm", name="warm")
            nc.vector.memset(warm[:], 0.0)
            for wi in range(WARM):
                pw = ph_pool.tile([P, TB], F32, tag="ph", name="pw")
                nc.tensor.matmul(pw[:], warm[:, :P], warm[:], start=True, stop=True)

            # resident tiles
            xt = [
                x_pool.tile([P, HB * TB], BF16, tag=f"x{b}", name=f"x{b}")
                for b in range(NB)
            ]
            w1at = [
                w1a_pool.tile([P, HB * P], BF16, tag=f"a{il}", name=f"a{il}")
                for il in range(GI)
            ]
            w2t = [
                w2_pool.tile([P, GI * H], BF16, tag=f"q{q}", name=f"q{q}")
                for q in range(IB // GI)
            ]

            def trig_x(b):
                # x rides the Scalar DGE ring: issues concurrently with the
                # w1 triggers on the Sync ring
                nc.scalar.dma_start(
                    out=xt[b][:], in_=xg[:, b * HB * TB : (b + 1) * HB * TB]
                )

            def trig_x_part(b, part, nparts):
                # k-chunks land separately so MM1 chains can start mid-delivery
                w = HB * TB // nparts
                o = b * HB * TB + part * w
                nc.scalar.dma_start(
                    out=xt[b][:, part * w : (part + 1) * w],
                    in_=xg[:, o : o + w],
                )

            def trig_w1a(il):
                nc.sync.dma_start(
                    out=w1at[il][:], in_=w1a[:, il * HB * P : (il + 1) * HB * P]
                )

            def trig_w1g(g):
                wt = w1_pool.tile([P, HB * TB], BF16, tag="wg", name="wg")
                nc.sync.dma_start(
                    out=wt[:], in_=w1b[:, (g - 1) * HB * TB : g * HB * TB]
                )
                return wt

            def trig_w1g0b():
                wt = w1a_pool.tile([P, GI * HB * P], BF16, tag="g0b", name="g0b")
                nc.sync.dma_start(out=wt[:], in_=w1a[:])
                return wt

            def trig_w2(q):
                nc.sync.dma_start(
                    out=w2t[q][:], in_=w2h[:, q * GI * H : (q + 1) * GI * H]
                )

            hT = {}

            def chain(b, i, lhsT_of_k):
                """One MM1 chain: hT[b%2, i] = gelu(w1[:, i-tile].T @ x[b])."""
                pht = ph_pool.tile([P, TB], F32, tag="ph", name="ph")
                for k in range(HB):
                    nc.tensor.matmul(
                        pht[:],
                        lhsT_of_k(k),
                        xt[b][:, k * TB : (k + 1) * TB],
                        start=(k == 0),
                        stop=(k == HB - 1),
                    )
                st = b % 2
                ht = h_pool.tile([P, TB], BF16, tag=f"h{st}_{i}", name=f"h{st}_{i}")
                nc.scalar.activation(ht[:], pht[:], GELU)
                hT[(st, i)] = ht

            def mm2_block(b, tail=False):
                st = b % 2
                for tc_ in range(TB // P):
                    pa = pyA_pool.tile([P, TB], F32, tag="pyA", name="pyA")
                    pb = pyB_pool.tile([P, TB], F32, tag="pyB", name="pyB")
                    for i in range(IB):
                        hs = hT[(st, i)][:, tc_ * P : (tc_ + 1) * P]
                        w2v = w2t[i // GI][:, (i % GI) * H : (i % GI + 1) * H]
                        nc.tensor.matmul(
                            pa[:], hs, w2v[:, : H // 2],
                            start=(i == 0), stop=(i == IB - 1),
                        )
                        nc.tensor.matmul(
                            pb[:], hs, w2v[:, H // 2 :],
                            start=(i == 0), stop=(i == IB - 1),
                        )
                    ysb = y_pool.tile([P, H], BF16, tag="yt", name="yt")
                    t0 = b * TB + tc_ * P
                    nc.scalar.activation(ysb[:, : H // 2], pa[:], COPY)
                    nc.sync.dma_start(
                        out=y[t0 : t0 + P, : H // 2], in_=ysb[:, : H // 2]
                    )
                    nc.scalar.activation(ysb[:, H // 2 :], pb[:], COPY)
                    # blocks 2+ store the second half via the Scalar DGE
                    # queue: spreads trigger load and keeps that queue warm
                    # for the tail's final store
                    dge = nc.scalar if b >= 2 else nc.sync
                    dge.dma_start(
                        out=y[t0 : t0 + P, H // 2 :], in_=ysb[:, H // 2 :]
                    )

            def mm2_block_tail(b):
                """Like mm2_block but the very last token tile splits its
                second output half into two 256-col PSUM chains so the
                final act+store after the last matmul is only 256 cols,
                drained by the Vector engine with the store issued from the
                Scalar DGE queue (both off the Sync queue's critical path)."""
                st = b % 2
                Q = H // 4
                for tc_ in range(TB // P):
                    last = tc_ == TB // P - 1
                    pa = pyA_pool.tile([P, TB], F32, tag="pyA", name="pyA")
                    pb = pyB_pool.tile([P, TB], F32, tag="pyB", name="pyB")
                    if last:
                        pc2 = ph_pool.tile([P, TB], F32, tag="ph", name="ph")
                    for i in range(IB):
                        hs = hT[(st, i)][:, tc_ * P : (tc_ + 1) * P]
                        w2v = w2t[i // GI][:, (i % GI) * H : (i % GI + 1) * H]
                        nc.tensor.matmul(
                            pa[:], hs, w2v[:, : H // 2],
                            start=(i == 0), stop=(i == IB - 1),
                        )
                        if last:
                            nc.tensor.matmul(
                                pb[:, :Q], hs, w2v[:, H // 2 : 3 * Q],
                                start=(i == 0), stop=(i == IB - 1),
                            )
                        else:
                            nc.tensor.matmul(
                                pb[:], hs, w2v[:, H // 2 :],
                                start=(i == 0), stop=(i == IB - 1),
                            )
                    if last:
                        # trailing 256-col chain: only its 64KB store follows
                        # the final matmul; pa/pb stores drain during it
                        for i in range(IB):
                            hs = hT[(st, i)][:, tc_ * P : (tc_ + 1) * P]
                            w2v = w2t[i // GI][:, (i % GI) * H : (i % GI + 1) * H]
                            nc.tensor.matmul(
                                pc2[:, :Q], hs, w2v[:, 3 * Q :],
                                start=(i == 0), stop=(i == IB - 1),
                            )
                    ysb = y_pool.tile([P, H], BF16, tag="yt", name="yt")
                    t0 = b * TB + tc_ * P
                    nc.scalar.activation(ysb[:, : H // 2], pa[:], COPY)
                    nc.sync.dma_start(
                        out=y[t0 : t0 + P, : H // 2], in_=ysb[:, : H // 2]
                    )
                    if last:
                        nc.vector.tensor_copy(ysb[:, H // 2 : 3 * Q], pb[:, :Q])
                        nc.sync.dma_start(
                            out=y[t0 : t0 + P, H // 2 : 3 * Q],
                            in_=ysb[:, H // 2 : 3 * Q],
                        )
                        nc.vector.tensor_copy(ysb[:, 3 * Q :], pc2[:, :Q])
                        nc.scalar.dma_start(
                            out=y[t0 : t0 + P, 3 * Q :], in_=ysb[:, 3 * Q :]
                        )
                    else:
                        nc.scalar.activation(ysb[:, H // 2 :], pb[:], COPY)
                        nc.scalar.dma_start(
                            out=y[t0 : t0 + P, H // 2 :], in_=ysb[:, H // 2 :]
                        )

            # ---- prologue DMA triggers (order = landing order per ring;
            # Sync carries w1, Scalar carries x, the rings run concurrently)
            trig_w1a(0)
            trig_x_part(0, 0, 2)
            trig_w1a(1)
            trig_x_part(0, 1, 2)
            trig_w1a(2)
            trig_w1a(3)
            trig_x_part(1, 0, 2)
            trig_x_part(1, 1, 2)
            w1t_cur = trig_w1g(1)

            # ---- pass 1 (blocks 0,1) ----
            # g0: single-block chains off the per-i-tile w1a tiles
            for b in (0, 1):
                for il in range(GI):
                    chain(b, il, lambda k, il=il: w1at[il][:, k * P : (k + 1) * P])
            # g1..g7: paired blocks off 1MB group tiles, prefetch g+1
            for g in range(1, NG):
                w1t_next = trig_w1g(g + 1) if g + 1 < NG else None
                if g == 2:
                    trig_w2(0)
                elif g == 3:
                    trig_w2(1)
                elif g == 4:
                    trig_w2(2)
                    trig_x(2)
                elif g == 5:
                    trig_w2(3)
                    trig_x(3)
                elif g == 6:
                    trig_w2(4)
                elif g == 7:
                    trig_w2(5)
                for b in (0, 1):
                    for il in range(GI):
                        i = g * GI + il
                        chain(
                            b,
                            i,
                            lambda k, il=il: w1t_cur[
                                :, k * TB + il * P : k * TB + (il + 1) * P
                            ],
                        )
                w1t_cur = w1t_next

            # prefetch pass-2 w1 (g0 packed tile + group 1) before the y DMAs
            # of mm2 enter the Sync queue
            w1g0b = trig_w1g0b()
            w1t_cur = trig_w1g(1)
            trig_w2(6)
            trig_w2(7)

            mm2_block(0)
            mm2_block(1)

            # ---- pass 2 (blocks 2,3) ----
            for b in (2, 3):
                for il in range(GI):
                    chain(
                        b,
                        il,
                        lambda k, il=il: w1g0b[
                            :, il * HB * P + k * P : il * HB * P + (k + 1) * P
                        ],
                    )
            for g in range(1, NG):
                w1t_next = trig_w1g(g + 1) if g + 1 < NG else None
                for b in (2, 3):
                    for il in range(GI):
                        i = g * GI + il
                        chain(
                            b,
                            i,
                            lambda k, il=il: w1t_cur[
                                :, k * TB + il * P : k * TB + (il + 1) * P
                            ],
                        )
                w1t_cur = w1t_next

            mm2_block(2)
            mm2_block_tail(3)

    nc.compile()
    return nc


def _get_nc():
    if "nc" not in _CACHE:
        _CACHE["nc"] = _build()
    return _CACHE["nc"]


def _prep_in_maps(x_sorted, w1, w2, expert_counts):
    counts = np.asarray(expert_counts, dtype=np.int64)
    n = x_sorted.shape[0]
    offsets = np.cumsum(counts)
    eid = np.searchsorted(offsets, np.arange(n), side="right")

    in_maps = []
    row_idx = []
    for e in range(E):
        rows = np.nonzero(eid == e)[0]
        assert len(rows) <= T, f"expert {e} overflows capacity {T}"
        xe = np.zeros((T, H), dtype=np.float32)
        xe[: len(rows)] = x_sorted[rows]
        row_idx.append(rows)

        # xg[p, b*HB*TB + k*TB + c] = xe[b*TB+c, k*P+p]
        xg = (
            xe.reshape(NB, TB, HB, P)
            .transpose(3, 0, 2, 1)
            .reshape(P, NB * HB * TB)
        )
        w1e = np.asarray(w1[e], dtype=np.float32)  # [H, I]
        # w1a[p, il*HB*P + k*P + c] = w1e[k*P+p, il*P+c]  (i-tiles 0..3)
        w1a = (
            w1e[:, : GI * P]
            .reshape(HB, P, GI, P)
            .transpose(1, 2, 0, 3)
            .reshape(P, GI * HB * P)
        )
        # w1b[p, g'*HB*TB + k*TB + c] = w1e[k*P+p, (g'+1)*TB+c]
        w1b = (
            w1e[:, TB:]
            .reshape(HB, P, NG - 1, TB)
            .transpose(1, 2, 0, 3)
            .reshape(P, (NG - 1) * HB * TB)
        )
        w2e = np.asarray(w2[e], dtype=np.float32)  # [I, H]
        # w2h[p, i*H + c] = w2e[i*P+p, c]
        w2h = w2e.reshape(IB, P, H).transpose(1, 0, 2).reshape(P, IB * H)

        in_maps.append(
            {
                "xg": np.ascontiguousarray(xg).astype(ml_dtypes.bfloat16),
                "w1a": np.ascontiguousarray(w1a).astype(ml_dtypes.bfloat16),
                "w1b": np.ascontiguousarray(w1b).astype(ml_dtypes.bfloat16),
                "w2h": np.ascontiguousarray(w2h).astype(ml_dtypes.bfloat16),
            }
        )
    return in_maps, row_idx


def kernel(x_sorted, w1, w2, expert_counts, local_expert_indices, **_unused):
    from concourse.bass_utils import run_bass_kernel_spmd

    x_sorted = np.ascontiguousarray(x_sorted, dtype=np.float32)
    nc = _get_nc()
    in_maps, row_idx = _prep_in_maps(x_sorted, w1, w2, expert_counts)
    res = run_bass_kernel_spmd(nc, in_maps, list(range(E))).results

    n = x_sorted.shape[0]
    out = np.zeros((n, H), dtype=np.float32)
    for e in range(E):
        rows = row_idx[e]
        out[rows] = np.asarray(res[e]["y"][: len(rows)], dtype=np.float32)
    return out


# revision 19
# speedup vs baseline: 1.0051x; 1.0051x over previous
"""MoE ExpertGroup kernel for Trainium2 (8 NeuronCores, expert-parallel).

Problem: E=8 experts, H=1024, I=4096, N=16384 tokens sorted by expert.
y[t] = gelu_tanh(x[t] @ w1[e(t)]) @ w2[e(t)]

Sharding: expert-parallel - core e holds expert e's weights and processes
expert e's token block (balanced routing: 2048 tokens/core). All matmul
operands are bf16 (full-rate on the PE, half the DMA traffic); y is
returned in bf16 and upcast on the host (rel err ~4e-3 vs 2e-2 budget).

Per-core structure (PE stream floor: 2048 matmuls x 512 cols @2.4GHz
= 442us; everything else is edges):
- Host pre-packs x/w1/w2 into k-major SBUF-shaped layouts so each DMA
  is ONE large contiguous-line transfer (the Sync queue issues triggers
  at only ~1.6/us, and each trigger fans out over all 16 DMA engines).
- MM1 pass 1 (blocks b0,b1): group g0 runs single-block chains (all b0,
  then all b1) with per-i-tile w1 triggers, so the first chain needs
  only 1.25MB landed (~12us) instead of 3MB (~18us). Groups g1..g7 run
  b0+b1 chains off one 1MB group tile, double buffered, prefetched one
  group ahead.
- MM2 accumulates all 32 I-tiles of a token tile into one PSUM group.
  y stored as bf16. For the very last token tile the second output half
  is drained by the Vector engine in parallel with the Scalar act to
  shorten the tail.
- 12 warmup matmuls ramp the PE clock (HAM un-throttle needs ~3.4us of
  busy) while the first DMAs land.
"""

import sys

sys.path.insert(0, "/opt/trn_rl_repo")

import numpy as np
import ml_dtypes

E = 8
H = 1024
I = 4096
N_TOK = 16384
T = N_TOK // E

P = 128
TB = 512
NB = T // TB          # 4 token blocks
HB = H // P           # 8 k-tiles (contraction for MM1)
IB = I // P           # 32 i-tiles
GI = 4                # i-tiles per w1 group
NG = IB // GI         # 8 groups
WARM = 11

_CACHE = {}


def _build():
    import concourse.bacc as bacc
    import concourse.mybir as mybir
    import concourse.tile as tile

    F32 = mybir.dt.float32
    BF16 = mybir.dt.bfloat16
    GELU = mybir.ActivationFunctionType.Gelu_apprx_tanh
    COPY = mybir.ActivationFunctionType.Copy

    nc = bacc.Bacc("TRN2", target_bir_lowering=False, debug=False, num_devices=E)

    # Host-packed layouts (see _prep_in_maps):
    #   xg [P, (b,k,c)]   : xg[p, b*HB*TB + k*TB + c] = x[b*TB+c, k*P+p]
    #   w1a [P, (il,k,c)] : w1a[p, il*HB*P + k*P + c] = w1[k*P+p, il*P+c]
    #   w1b [P, (g',k,c)] : w1b[p, g'*HB*TB + k*TB + c] = w1[k*P+p, (g'+1)*TB+c]
    #   w2h [P, (i,c)]    : w2h[p, i*H + c] = w2[i*P+p, c]
    xg = nc.dram_tensor("xg", [P, NB * HB * TB], BF16, kind="ExternalInput").ap()
    w1a = nc.dram_tensor("w1a", [P, GI * HB * P], BF16, kind="ExternalInput").ap()
    w1b = nc.dram_tensor(
        "w1b", [P, (NG - 1) * HB * TB], BF16, kind="ExternalInput"
    ).ap()
    w2h = nc.dram_tensor("w2h", [P, IB * H], BF16, kind="ExternalInput").ap()
    y = nc.dram_tensor("y", [T, H], BF16, kind="ExternalOutput").ap()

    with tile.TileContext(nc) as tc:
        with (
            tc.tile_pool(name="xp", bufs=1) as x_pool,
            tc.tile_pool(name="w1ap", bufs=1) as w1a_pool,
            tc.tile_pool(name="w1p", bufs=2) as w1_pool,
            tc.tile_pool(name="w2p", bufs=1) as w2_pool,
            tc.tile_pool(name="hp", bufs=1) as h_pool,
            tc.tile_pool(name="yp", bufs=4) as y_pool,
            tc.tile_pool(name="ph", bufs=4, space="PSUM") as ph_pool,
            tc.tile_pool(name="pyA", bufs=2, space="PSUM") as pyA_pool,
            tc.tile_pool(name="pyB", bufs=2, space="PSUM") as pyB_pool,
        ):
            # warmup first: PE busy while the first DMAs land (HAM ramp)
            warm = y_pool.tile([P, TB], BF16, tag="warm", name="warm")
            nc.vector.memset(warm[:], 0.0)
            for wi in range(WARM):
                pw = ph_pool.tile([P, TB], F32, tag="ph", name="pw")
                nc.tensor.matmul(pw[:], warm[:, :P], warm[:], start=True, stop=True)

            # resident tiles
            xt = [
                x_pool.tile([P, HB * TB], BF16, tag=f"x{b}", name=f"x{b}")
                for b in range(NB)
            ]
            w1at = [
                w1a_pool.tile([P, HB * P], BF16, tag=f"a{il}", name=f"a{il}")
                for il in range(GI)
            ]
            w2t = [
                w2_pool.tile([P, GI * H], BF16, tag=f"q{q}", name=f"q{q}")
                for q in range(IB // GI)
            ]

            def trig_x(b):
                nc.sync.dma_start(
                    out=xt[b][:], in_=xg[:, b * HB * TB : (b + 1) * HB * TB]
                )

            def trig_x_span(b, c0, c1):
                # k-chunks land separately so MM1 chains can start mid-delivery
                o = b * HB * TB
                nc.sync.dma_start(
                    out=xt[b][:, c0:c1], in_=xg[:, o + c0 : o + c1]
                )

            def trig_w1a(il):
                nc.sync.dma_start(
                    out=w1at[il][:], in_=w1a[:, il * HB * P : (il + 1) * HB * P]
                )

            def trig_w1g(g):
                wt = w1_pool.tile([P, HB * TB], BF16, tag="wg", name="wg")
                nc.sync.dma_start(
                    out=wt[:], in_=w1b[:, (g - 1) * HB * TB : g * HB * TB]
                )
                return wt

            def trig_w1g0b():
                wt = w1a_pool.tile([P, GI * HB * P], BF16, tag="g0b", name="g0b")
                nc.sync.dma_start(out=wt[:], in_=w1a[:])
                return wt

            def trig_w2(q):
                nc.sync.dma_start(
                    out=w2t[q][:], in_=w2h[:, q * GI * H : (q + 1) * GI * H]
                )

            hT = {}

            def chain(b, i, lhsT_of_k):
                """One MM1 chain: hT[b%2, i] = gelu(w1[:, i-tile].T @ x[b])."""
                pht = ph_pool.tile([P, TB], F32, tag="ph", name="ph")
                for k in range(HB):
                    nc.tensor.matmul(
                        pht[:],
                        lhsT_of_k(k),
                        xt[b][:, k * TB : (k + 1) * TB],
                        start=(k == 0),
                        stop=(k == HB - 1),
                    )
                st = b % 2
                ht = h_pool.tile([P, TB], BF16, tag=f"h{st}_{i}", name=f"h{st}_{i}")
                nc.scalar.activation(ht[:], pht[:], GELU)
                hT[(st, i)] = ht

            def mm2_block(b, tail=False):
                st = b % 2
                for tc_ in range(TB // P):
                    pa = pyA_pool.tile([P, TB], F32, tag="pyA", name="pyA")
                    pb = pyB_pool.tile([P, TB], F32, tag="pyB", name="pyB")
                    for i in range(IB):
                        hs = hT[(st, i)][:, tc_ * P : (tc_ + 1) * P]
                        w2v = w2t[i // GI][:, (i % GI) * H : (i % GI + 1) * H]
                        nc.tensor.matmul(
                            pa[:], hs, w2v[:, : H // 2],
                            start=(i == 0), stop=(i == IB - 1),
                        )
                        nc.tensor.matmul(
                            pb[:], hs, w2v[:, H // 2 :],
                            start=(i == 0), stop=(i == IB - 1),
                        )
                    ysb = y_pool.tile([P, H], BF16, tag="yt", name="yt")
                    t0 = b * TB + tc_ * P
                    nc.scalar.activation(ysb[:, : H // 2], pa[:], COPY)
                    nc.sync.dma_start(
                        out=y[t0 : t0 + P, : H // 2], in_=ysb[:, : H // 2]
                    )
                    nc.scalar.activation(ysb[:, H // 2 :], pb[:], COPY)
                    # blocks 2+ store the second half via the Scalar DGE
                    # queue: spreads trigger load and keeps that queue warm
                    # for the tail's final store
                    dge = nc.scalar if b >= 2 else nc.sync
                    dge.dma_start(
                        out=y[t0 : t0 + P, H // 2 :], in_=ysb[:, H // 2 :]
                    )

            def mm2_block_tail(b):
                """Like mm2_block but the very last token tile splits its
                second output half into two 256-col PSUM chains so the
                final act+store after the last matmul is only 256 cols,
                drained by the Vector engine with the store issued from the
                Scalar DGE queue (both off the Sync queue's critical path)."""
                st = b % 2
                Q = H // 4
                for tc_ in range(TB // P):
                    last = tc_ == TB // P - 1
                    pa = pyA_pool.tile([P, TB], F32, tag="pyA", name="pyA")
                    pb = pyB_pool.tile([P, TB], F32, tag="pyB", name="pyB")
                    if last:
                        pc2 = ph_pool.tile([P, TB], F32, tag="ph", name="ph")
                    for i in range(IB):
                        hs = hT[(st, i)][:, tc_ * P : (tc_ + 1) * P]
                        w2v = w2t[i // GI][:, (i % GI) * H : (i % GI + 1) * H]
                        nc.tensor.matmul(
                            pa[:], hs, w2v[:, : H // 2],
                            start=(i == 0), stop=(i == IB - 1),
                        )
                        if last:
                            nc.tensor.matmul(
                                pb[:, :Q], hs, w2v[:, H // 2 : 3 * Q],
                                start=(i == 0), stop=(i == IB - 1),
                            )
                        else:
                            nc.tensor.matmul(
                                pb[:], hs, w2v[:, H // 2 :],
                                start=(i == 0), stop=(i == IB - 1),
                            )
                    if last:
                        # trailing 256-col chain: only its 64KB store follows
                        # the final matmul; pa/pb stores drain during it
                        for i in range(IB):
                            hs = hT[(st, i)][:, tc_ * P : (tc_ + 1) * P]
                            w2v = w2t[i // GI][:, (i % GI) * H : (i % GI + 1) * H]
                            nc.tensor.matmul(
                                pc2[:, :Q], hs, w2v[:, 3 * Q :],
                                start=(i == 0), stop=(i == IB - 1),
                            )
                    ysb = y_pool.tile([P, H], BF16, tag="yt", name="yt")
                    t0 = b * TB + tc_ * P
                    nc.scalar.activation(ysb[:, : H // 2], pa[:], COPY)
                    nc.sync.dma_start(
                        out=y[t0 : t0 + P, : H // 2], in_=ysb[:, : H // 2]
                    )
                    if last:
                        nc.vector.tensor_copy(ysb[:, H // 2 : 3 * Q], pb[:, :Q])
                        nc.sync.dma_start(
                            out=y[t0 : t0 + P, H // 2 : 3 * Q],
                            in_=ysb[:, H // 2 : 3 * Q],
                        )
                        nc.vector.tensor_copy(ysb[:, 3 * Q :], pc2[:, :Q])
                        nc.scalar.dma_start(
                            out=y[t0 : t0 + P, 3 * Q :], in_=ysb[:, 3 * Q :]
                        )
                    else:
                        nc.scalar.activation(ysb[:, H // 2 :], pb[:], COPY)
                        nc.scalar.dma_start(
                            out=y[t0 : t0 + P, H // 2 :], in_=ysb[:, H // 2 :]
                        )

            # ---- prologue DMA triggers (order = landing order) ----
            # x0 split 2k+6k: the first chain starts on a small early chunk
            trig_w1a(0)
            trig_x_span(0, 0, 2 * TB)
            trig_x_span(0, 2 * TB, HB * TB)
            trig_w1a(1)
            trig_w1a(2)
            trig_w1a(3)
            trig_x_span(1, 0, 4 * TB)
            trig_x_span(1, 4 * TB, HB * TB)
            w1t_cur = trig_w1g(1)

            # ---- pass 1 (blocks 0,1) ----
            # g0: single-block chains off the per-i-tile w1a tiles
            for b in (0, 1):
                for il in range(GI):
                    chain(b, il, lambda k, il=il: w1at[il][:, k * P : (k + 1) * P])
            # g1..g7: paired blocks off 1MB group tiles, prefetch g+1
            for g in range(1, NG):
                w1t_next = trig_w1g(g + 1) if g + 1 < NG else None
                if g == 2:
                    trig_w2(0)
                elif g == 3:
                    trig_w2(1)
                elif g == 4:
                    trig_w2(2)
                    trig_x(2)
                elif g == 5:
                    trig_w2(3)
                    trig_x(3)
                elif g == 6:
                    trig_w2(4)
                elif g == 7:
                    trig_w2(5)
                for b in (0, 1):
                    for il in range(GI):
                        i = g * GI + il
                        chain(
                            b,
                            i,
                            lambda k, il=il: w1t_cur[
                                :, k * TB + il * P : k * TB + (il + 1) * P
                            ],
                        )
                w1t_cur = w1t_next

            # prefetch pass-2 w1 (g0 packed tile + group 1) before the y DMAs
            # of mm2 enter the Sync queue
            w1g0b = trig_w1g0b()
            w1t_cur = trig_w1g(1)
            trig_w2(6)
            trig_w2(7)

            mm2_block(0)
            mm2_block(1)

            # ---- pass 2 (blocks 2,3) ----
            for b in (2, 3):
                for il in range(GI):
                    chain(
                        b,
                        il,
                        lambda k, il=il: w1g0b[
                            :, il * HB * P + k * P : il * HB * P + (k + 1) * P
                        ],
                    )
            for g in range(1, NG):
                w1t_next = trig_w1g(g + 1) if g + 1 < NG else None
                for b in (2, 3):
                    for il in range(GI):
                        i = g * GI + il
                        chain(
                            b,
                            i,
                            lambda k, il=il: w1t_cur[
                                :, k * TB + il * P : k * TB + (il + 1) * P
                            ],
                        )
                w1t_cur = w1t_next

            mm2_block(2)
            mm2_block_tail(3)

    nc.compile()
    return nc


def _get_nc():
    if "nc" not in _CACHE:
        _CACHE["nc"] = _build()
    return _CACHE["nc"]


def _prep_in_maps(x_sorted, w1, w2, expert_counts):
    counts = np.asarray(expert_counts, dtype=np.int64)
    n = x_sorted.shape[0]
    offsets = np.cumsum(counts)
    eid = np.searchsorted(offsets, np.arange(n), side="right")

    in_maps = []
    row_idx = []
    for e in range(E):
        rows = np.nonzero(eid == e)[0]
        assert len(rows) <= T, f"expert {e} overflows capacity {T}"
        xe = np.zeros((T, H), dtype=np.float32)
        xe[: len(rows)] = x_sorted[rows]
        row_idx.append(rows)

        # xg[p, b*HB*TB + k*TB + c] = xe[b*TB+c, k*P+p]
        xg = (
            xe.reshape(NB, TB, HB, P)
            .transpose(3, 0, 2, 1)
            .reshape(P, NB * HB * TB)
        )
        w1e = np.asarray(w1[e], dtype=np.float32)  # [H, I]
        # w1a[p, il*HB*P + k*P + c] = w1e[k*P+p, il*P+c]  (i-tiles 0..3)
        w1a = (
            w1e[:, : GI * P]
            .reshape(HB, P, GI, P)
            .transpose(1, 2, 0, 3)
            .reshape(P, GI * HB * P)
        )
        # w1b[p, g'*HB*TB + k*TB + c] = w1e[k*P+p, (g'+1)*TB+c]
        w1b = (
            w1e[:, TB:]
            .reshape(HB, P, NG - 1, TB)
            .transpose(1, 2, 0, 3)
            .reshape(P, (NG - 1) * HB * TB)
        )
        w2e = np.asarray(w2[e], dtype=np.float32)  # [I, H]
        # w2h[p, i*H + c] = w2e[i*P+p, c]
        w2h = w2e.reshape(IB, P, H).transpose(1, 0, 2).reshape(P, IB * H)

        in_maps.append(
            {
                "xg": np.ascontiguousarray(xg).astype(ml_dtypes.bfloat16),
                "w1a": np.ascontiguousarray(w1a).astype(ml_dtypes.bfloat16),
                "w1b": np.ascontiguousarray(w1b).astype(ml_dtypes.bfloat16),
                "w2h": np.ascontiguousarray(w2h).astype(ml_dtypes.bfloat16),
            }
        )
    return in_maps, row_idx


def kernel(x_sorted, w1, w2, expert_counts, local_expert_indices, **_unused):
    from concourse.bass_utils import run_bass_kernel_spmd

    x_sorted = np.ascontiguousarray(x_sorted, dtype=np.float32)
    nc = _get_nc()
    in_maps, row_idx = _prep_in_maps(x_sorted, w1, w2, expert_counts)
    res = run_bass_kernel_spmd(nc, in_maps, list(range(E))).results

    n = x_sorted.shape[0]
    out = np.zeros((n, H), dtype=np.float32)
    for e in range(E):
        rows = row_idx[e]
        out[rows] = np.asarray(res[e]["y"][: len(rows)], dtype=np.float32)
    return out


# revision 29
# speedup vs baseline: 1.0336x; 1.0284x over previous
"""MoE ExpertGroup kernel for Trainium2 (8 NeuronCores, expert-parallel).

Problem: E=8 experts, H=1024, I=4096, N=16384 tokens sorted by expert.
y[t] = gelu_tanh(x[t] @ w1[e(t)]) @ w2[e(t)]

Sharding: expert-parallel - core e holds expert e's weights and processes
expert e's token block (balanced routing: 2048 tokens/core). All matmul
operands are bf16 (full-rate on the PE, half the DMA traffic); y is
returned in bf16 and upcast on the host (rel err ~4e-3 vs 2e-2 budget).

Per-core structure (PE stream floor: 2048 matmuls x 512 cols @2.4GHz
= 442us; everything else is edges):
- Host pre-packs x/w1/w2 into k-major SBUF-shaped layouts so each DMA
  is ONE large contiguous-line transfer (the Sync queue issues triggers
  at only ~1.6/us, and each trigger fans out over all 16 DMA engines).
- MM1 pass 1 (blocks b0,b1): group g0 runs single-block chains (all b0,
  then all b1) with per-i-tile w1 triggers, so the first chain needs
  only 1.25MB landed (~12us) instead of 3MB (~18us). Groups g1..g7 run
  b0+b1 chains off one 1MB group tile, double buffered, prefetched one
  group ahead.
- MM2 accumulates all 32 I-tiles of a token tile into one PSUM group.
  y stored as bf16. For the very last token tile the second output half
  is drained by the Vector engine in parallel with the Scalar act to
  shorten the tail.
- 12 warmup matmuls ramp the PE clock (HAM un-throttle needs ~3.4us of
  busy) while the first DMAs land.
"""

import sys

sys.path.insert(0, "/opt/trn_rl_repo")

import numpy as np
import ml_dtypes

E = 8
H = 1024
I = 4096
N_TOK = 16384
T = N_TOK // E

P = 128
TB = 512
NB = T // TB          # 4 token blocks
HB = H // P           # 8 k-tiles (contraction for MM1)
IB = I // P           # 32 i-tiles
GI = 4                # i-tiles per w1 group
NG = IB // GI         # 8 groups
WARM = 11
F8 = 2                # i-tile PAIRS of MM2 run in fp8 DoubleRow (2x PE rate)
I8_START = IB - 2 * F8  # first fp8 i-tile

_CACHE = {}


def _build():
    import concourse.bacc as bacc
    import concourse.mybir as mybir
    import concourse.tile as tile

    F32 = mybir.dt.float32
    BF16 = mybir.dt.bfloat16
    FP8 = mybir.dt.float8e4
    DR = mybir.MatmulPerfMode.DoubleRow
    GELU = mybir.ActivationFunctionType.Gelu_apprx_tanh
    COPY = mybir.ActivationFunctionType.Copy

    nc = bacc.Bacc("TRN2", target_bir_lowering=False, debug=False, num_devices=E)

    # Host-packed layouts (see _prep_in_maps):
    #   xg [P, (b,k,c)]   : xg[p, b*HB*TB + k*TB + c] = x[b*TB+c, k*P+p]
    #   w1a [P, (il,k,c)] : w1a[p, il*HB*P + k*P + c] = w1[k*P+p, il*P+c]
    #   w1b [P, (g',k,c)] : w1b[p, g'*HB*TB + k*TB + c] = w1[k*P+p, (g'+1)*TB+c]
    #   w2h [P, (i,c)]    : w2h[p, i*H + c] = w2[i*P+p, c]
    xg = nc.dram_tensor("xg", [P, NB * HB * TB], BF16, kind="ExternalInput").ap()
    w1a = nc.dram_tensor("w1a", [P, GI * HB * P], BF16, kind="ExternalInput").ap()
    w1b = nc.dram_tensor(
        "w1b", [P, (NG - 1) * HB * TB], BF16, kind="ExternalInput"
    ).ap()
    w2h = nc.dram_tensor("w2h", [P, IB * H], BF16, kind="ExternalInput").ap()
    # fp8 copies of the last 2*F8 i-tiles of w2, packed (q, ko, n)
    w2f8 = nc.dram_tensor("w2f8", [P, F8 * 2 * H], FP8, kind="ExternalInput").ap()
    y = nc.dram_tensor("y", [T, H], BF16, kind="ExternalOutput").ap()

    with tile.TileContext(nc) as tc:
        with (
            tc.tile_pool(name="xp", bufs=1) as x_pool,
            tc.tile_pool(name="w1ap", bufs=1) as w1a_pool,
            tc.tile_pool(name="w1p", bufs=2) as w1_pool,
            tc.tile_pool(name="w2p", bufs=1) as w2_pool,
            tc.tile_pool(name="hp", bufs=1) as h_pool,
            tc.tile_pool(name="yp", bufs=4) as y_pool,
            tc.tile_pool(name="ph", bufs=4, space="PSUM") as ph_pool,
            tc.tile_pool(name="pyA", bufs=2, space="PSUM") as pyA_pool,
            tc.tile_pool(name="pyB", bufs=2, space="PSUM") as pyB_pool,
        ):
            # warmup first: PE busy while the first DMAs land (HAM ramp)
            warm = y_pool.tile([P, TB], BF16, tag="warm", name="warm")
            nc.vector.memset(warm[:], 0.0)
            for wi in range(WARM):
                pw = ph_pool.tile([P, TB], F32, tag="ph", name="pw")
                nc.tensor.matmul(pw[:], warm[:, :P], warm[:], start=True, stop=True)

            # resident tiles
            xt = [
                x_pool.tile([P, HB * TB], BF16, tag=f"x{b}", name=f"x{b}")
                for b in range(NB)
            ]
            w1at = [
                w1a_pool.tile([P, HB * P], BF16, tag=f"a{il}", name=f"a{il}")
                for il in range(GI)
            ]
            w2t = [
                w2_pool.tile([P, GI * H], BF16, tag=f"q{q}", name=f"q{q}")
                for q in range(IB // GI)
            ]
            w2f8t = [
                w2_pool.tile([P, 2, H], FP8, tag=f"f8_{q}", name=f"f8_{q}")
                for q in range(F8)
            ]

            def trig_x(b):
                nc.sync.dma_start(
                    out=xt[b][:], in_=xg[:, b * HB * TB : (b + 1) * HB * TB]
                )

            def trig_x_span(b, c0, c1):
                # k-chunks land separately so MM1 chains can start mid-delivery
                o = b * HB * TB
                nc.sync.dma_start(
                    out=xt[b][:, c0:c1], in_=xg[:, o + c0 : o + c1]
                )

            def trig_w1a(il):
                nc.sync.dma_start(
                    out=w1at[il][:], in_=w1a[:, il * HB * P : (il + 1) * HB * P]
                )

            def trig_w1g(g):
                wt = w1_pool.tile([P, HB * TB], BF16, tag="wg", name="wg")
                nc.sync.dma_start(
                    out=wt[:], in_=w1b[:, (g - 1) * HB * TB : g * HB * TB]
                )
                return wt

            def trig_w1g0b():
                wt = w1a_pool.tile([P, GI * HB * P], BF16, tag="g0b", name="g0b")
                nc.sync.dma_start(out=wt[:], in_=w1a[:])
                return wt

            def trig_w2(q):
                nc.sync.dma_start(
                    out=w2t[q][:], in_=w2h[:, q * GI * H : (q + 1) * GI * H]
                )

            def trig_w2f8(q):
                nc.sync.dma_start(
                    out=w2f8t[q][:], in_=w2f8[:, q * 2 * H : (q + 1) * 2 * H]
                )

            hT = {}
            h8T = {}

            def chain(b, i, lhsT_of_k):
                """One MM1 chain: hT[b%2, i] = gelu(w1[:, i-tile].T @ x[b])."""
                pht = ph_pool.tile([P, TB], F32, tag="ph", name="ph")
                for k in range(HB):
                    nc.tensor.matmul(
                        pht[:],
                        lhsT_of_k(k),
                        xt[b][:, k * TB : (k + 1) * TB],
                        start=(k == 0),
                        stop=(k == HB - 1),
                    )
                st = b % 2
                if i >= I8_START:
                    # fp8 pair tile [P, 2, TB]: ko = i%2 selects the half
                    q = (i - I8_START) // 2
                    if i % 2 == 0:
                        h8T[(st, q)] = h_pool.tile(
                            [P, 2, TB], FP8, tag=f"h8_{st}_{q}", name=f"h8_{st}_{q}"
                        )
                    nc.scalar.activation(h8T[(st, q)][:, i % 2, :], pht[:], GELU)
                else:
                    ht = h_pool.tile(
                        [P, TB], BF16, tag=f"h{st}_{i}", name=f"h{st}_{i}"
                    )
                    nc.scalar.activation(ht[:], pht[:], GELU)
                    hT[(st, i)] = ht

            def mm2_block(b, tail=False):
                st = b % 2
                for tc_ in range(TB // P):
                    pa = pyA_pool.tile([P, TB], F32, tag="pyA", name="pyA")
                    pb = pyB_pool.tile([P, TB], F32, tag="pyB", name="pyB")
                    for i in range(I8_START):
                        hs = hT[(st, i)][:, tc_ * P : (tc_ + 1) * P]
                        w2v = w2t[i // GI][:, (i % GI) * H : (i % GI + 1) * H]
                        nc.tensor.matmul(
                            pa[:], hs, w2v[:, : H // 2],
                            start=(i == 0), stop=False,
                        )
                        nc.tensor.matmul(
                            pb[:], hs, w2v[:, H // 2 :],
                            start=(i == 0), stop=False,
                        )
                    for q in range(F8):
                        hs8 = h8T[(st, q)][:, :, tc_ * P : (tc_ + 1) * P]
                        nc.tensor.matmul(
                            pa[:], hs8, w2f8t[q][:, :, : H // 2],
                            start=False, stop=(q == F8 - 1), perf_mode=DR,
                        )
                        nc.tensor.matmul(
                            pb[:], hs8, w2f8t[q][:, :, H // 2 :],
                            start=False, stop=(q == F8 - 1), perf_mode=DR,
                        )
                    ysb = y_pool.tile([P, H], BF16, tag="yt", name="yt")
                    t0 = b * TB + tc_ * P
                    nc.scalar.activation(ysb[:, : H // 2], pa[:], COPY)
                    nc.sync.dma_start(
                        out=y[t0 : t0 + P, : H // 2], in_=ysb[:, : H // 2]
                    )
                    nc.scalar.activation(ysb[:, H // 2 :], pb[:], COPY)
                    # blocks 2+ store the second half via the Scalar DGE
                    # queue: spreads trigger load and keeps that queue warm
                    # for the tail's final store
                    dge = nc.scalar if b >= 2 else nc.sync
                    dge.dma_start(
                        out=y[t0 : t0 + P, H // 2 :], in_=ysb[:, H // 2 :]
                    )

            def mm2_block_tail(b):
                """Like mm2_block but the very last token tile splits its
                second output half into two 256-col PSUM chains so the
                final act+store after the last matmul is only 256 cols,
                drained by the Vector engine with the store issued from the
                Scalar DGE queue (both off the Sync queue's critical path)."""
                st = b % 2
                Q = H // 4
                for tc_ in range(TB // P):
                    last = tc_ == TB // P - 1
                    pa = pyA_pool.tile([P, TB], F32, tag="pyA", name="pyA")
                    pb = pyB_pool.tile([P, TB], F32, tag="pyB", name="pyB")
                    if last:
                        pc2 = ph_pool.tile([P, TB], F32, tag="ph", name="ph")
                    for i in range(I8_START):
                        hs = hT[(st, i)][:, tc_ * P : (tc_ + 1) * P]
                        w2v = w2t[i // GI][:, (i % GI) * H : (i % GI + 1) * H]
                        nc.tensor.matmul(
                            pa[:], hs, w2v[:, : H // 2],
                            start=(i == 0), stop=False,
                        )
                        if last:
                            nc.tensor.matmul(
                                pb[:, :Q], hs, w2v[:, H // 2 : 3 * Q],
                                start=(i == 0), stop=False,
                            )
                        else:
                            nc.tensor.matmul(
                                pb[:], hs, w2v[:, H // 2 :],
                                start=(i == 0), stop=False,
                            )
                    for q in range(F8):
                        hs8 = h8T[(st, q)][:, :, tc_ * P : (tc_ + 1) * P]
                        nc.tensor.matmul(
                            pa[:], hs8, w2f8t[q][:, :, : H // 2],
                            start=False, stop=(q == F8 - 1), perf_mode=DR,
                        )
                        if last:
                            nc.tensor.matmul(
                                pb[:, :Q], hs8, w2f8t[q][:, :, H // 2 : 3 * Q],
                                start=False, stop=(q == F8 - 1), perf_mode=DR,
                            )
                        else:
                            nc.tensor.matmul(
                                pb[:], hs8, w2f8t[q][:, :, H // 2 :],
                                start=False, stop=(q == F8 - 1), perf_mode=DR,
                            )
                    if last:
                        # trailing 256-col chain: only its 64KB store follows
                        # the final matmul; pa/pb stores drain during it
                        for i in range(I8_START):
                            hs = hT[(st, i)][:, tc_ * P : (tc_ + 1) * P]
                            w2v = w2t[i // GI][:, (i % GI) * H : (i % GI + 1) * H]
                            nc.tensor.matmul(
                                pc2[:, :Q], hs, w2v[:, 3 * Q :],
                                start=(i == 0), stop=False,
                            )
                        for q in range(F8):
                            hs8 = h8T[(st, q)][:, :, tc_ * P : (tc_ + 1) * P]
                            nc.tensor.matmul(
                                pc2[:, :Q], hs8, w2f8t[q][:, :, 3 * Q :],
                                start=False, stop=(q == F8 - 1), perf_mode=DR,
                            )
                    ysb = y_pool.tile([P, H], BF16, tag="yt", name="yt")
                    t0 = b * TB + tc_ * P
                    nc.scalar.activation(ysb[:, : H // 2], pa[:], COPY)
                    nc.sync.dma_start(
                        out=y[t0 : t0 + P, : H // 2], in_=ysb[:, : H // 2]
                    )
                    if last:
                        nc.vector.tensor_copy(ysb[:, H // 2 : 3 * Q], pb[:, :Q])
                        nc.sync.dma_start(
                            out=y[t0 : t0 + P, H // 2 : 3 * Q],
                            in_=ysb[:, H // 2 : 3 * Q],
                        )
                        nc.vector.tensor_copy(ysb[:, 3 * Q :], pc2[:, :Q])
                        nc.scalar.dma_start(
                            out=y[t0 : t0 + P, 3 * Q :], in_=ysb[:, 3 * Q :]
                        )
                    else:
                        nc.scalar.activation(ysb[:, H // 2 :], pb[:], COPY)
                        nc.scalar.dma_start(
                            out=y[t0 : t0 + P, H // 2 :], in_=ysb[:, H // 2 :]
                        )

            # ---- prologue DMA triggers (order = landing order) ----
            # x0 split 2k+6k: the first chain starts on a small early chunk
            trig_w1a(0)
            trig_x_span(0, 0, 2 * TB)
            trig_x_span(0, 2 * TB, HB * TB)
            trig_w1a(1)
            trig_w1a(2)
            trig_w1a(3)
            trig_x_span(1, 0, 4 * TB)
            trig_x_span(1, 4 * TB, HB * TB)
            w1t_cur = trig_w1g(1)

            # ---- pass 1 (blocks 0,1) ----
            # g0: single-block chains off the per-i-tile w1a tiles
            for b in (0, 1):
                for il in range(GI):
                    chain(b, il, lambda k, il=il: w1at[il][:, k * P : (k + 1) * P])
            # g1..g7: paired blocks off 1MB group tiles, prefetch g+1
            for g in range(1, NG):
                w1t_next = trig_w1g(g + 1) if g + 1 < NG else None
                if g == 2:
                    trig_w2(0)
                elif g == 3:
                    trig_w2(1)
                elif g == 4:
                    trig_w2(2)
                    trig_x(2)
                elif g == 5:
                    trig_w2(3)
                    trig_x(3)
                elif g == 6:
                    trig_w2(4)
                elif g == 7:
                    trig_w2(5)
                for b in (0, 1):
                    for il in range(GI):
                        i = g * GI + il
                        chain(
                            b,
                            i,
                            lambda k, il=il: w1t_cur[
                                :, k * TB + il * P : k * TB + (il + 1) * P
                            ],
                        )
                w1t_cur = w1t_next

            # prefetch pass-2 w1 (g0 packed tile + group 1) before the y DMAs
            # of mm2 enter the Sync queue
            w1g0b = trig_w1g0b()
            w1t_cur = trig_w1g(1)
            trig_w2(6)
            trig_w2(7)
            for q in range(F8):
                trig_w2f8(q)

            mm2_block(0)
            mm2_block(1)

            # ---- pass 2 (blocks 2,3) ----
            for b in (2, 3):
                for il in range(GI):
                    chain(
                        b,
                        il,
                        lambda k, il=il: w1g0b[
                            :, il * HB * P + k * P : il * HB * P + (k + 1) * P
                        ],
                    )
            for g in range(1, NG):
                w1t_next = trig_w1g(g + 1) if g + 1 < NG else None
                for b in (2, 3):
                    for il in range(GI):
                        i = g * GI + il
                        chain(
                            b,
                            i,
                            lambda k, il=il: w1t_cur[
                                :, k * TB + il * P : k * TB + (il + 1) * P
                            ],
                        )
                w1t_cur = w1t_next

            mm2_block(2)
            mm2_block_tail(3)

    nc.compile()
    return nc


def _get_nc():
    if "nc" not in _CACHE:
        _CACHE["nc"] = _build()
    return _CACHE["nc"]


def _prep_in_maps(x_sorted, w1, w2, expert_counts):
    counts = np.asarray(expert_counts, dtype=np.int64)
    n = x_sorted.shape[0]
    offsets = np.cumsum(counts)
    eid = np.searchsorted(offsets, np.arange(n), side="right")

    in_maps = []
    row_idx = []
    for e in range(E):
        rows = np.nonzero(eid == e)[0]
        assert len(rows) <= T, f"expert {e} overflows capacity {T}"
        xe = np.zeros((T, H), dtype=np.float32)
        xe[: len(rows)] = x_sorted[rows]
        row_idx.append(rows)

        # xg[p, b*HB*TB + k*TB + c] = xe[b*TB+c, k*P+p]
        xg = (
            xe.reshape(NB, TB, HB, P)
            .transpose(3, 0, 2, 1)
            .reshape(P, NB * HB * TB)
        )
        w1e = np.asarray(w1[e], dtype=np.float32)  # [H, I]
        # w1a[p, il*HB*P + k*P + c] = w1e[k*P+p, il*P+c]  (i-tiles 0..3)
        w1a = (
            w1e[:, : GI * P]
            .reshape(HB, P, GI, P)
            .transpose(1, 2, 0, 3)
            .reshape(P, GI * HB * P)
        )
        # w1b[p, g'*HB*TB + k*TB + c] = w1e[k*P+p, (g'+1)*TB+c]
        w1b = (
            w1e[:, TB:]
            .reshape(HB, P, NG - 1, TB)
            .transpose(1, 2, 0, 3)
            .reshape(P, (NG - 1) * HB * TB)
        )
        w2e = np.asarray(w2[e], dtype=np.float32)  # [I, H]
        # w2h[p, i*H + c] = w2e[i*P+p, c]
        w2h = w2e.reshape(IB, P, H).transpose(1, 0, 2).reshape(P, IB * H)
        # w2f8[p, (2q+ko)*H + n] = w2e[(I8_START+2q+ko)*P + p, n]
        w2f8 = (
            w2e[I8_START * P :]
            .reshape(2 * F8, P, H)
            .transpose(1, 0, 2)
            .reshape(P, 2 * F8 * H)
        )

        in_maps.append(
            {
                "xg": np.ascontiguousarray(xg).astype(ml_dtypes.bfloat16),
                "w1a": np.ascontiguousarray(w1a).astype(ml_dtypes.bfloat16),
                "w1b": np.ascontiguousarray(w1b).astype(ml_dtypes.bfloat16),
                "w2h": np.ascontiguousarray(w2h).astype(ml_dtypes.bfloat16),
                "w2f8": np.ascontiguousarray(w2f8).astype(ml_dtypes.float8_e4m3),
            }
        )
    return in_maps, row_idx


def kernel(x_sorted, w1, w2, expert_counts, local_expert_indices, **_unused):
    from concourse.bass_utils import run_bass_kernel_spmd

    x_sorted = np.ascontiguousarray(x_sorted, dtype=np.float32)
    nc = _get_nc()
    in_maps, row_idx = _prep_in_maps(x_sorted, w1, w2, expert_counts)
    res = run_bass_kernel_spmd(nc, in_maps, list(range(E))).results

    n = x_sorted.shape[0]
    out = np.zeros((n, H), dtype=np.float32)
    for e in range(E):
        rows = row_idx[e]
        out[rows] = np.asarray(res[e]["y"][: len(rows)], dtype=np.float32)
    return out
